# revision 29
# baseline (speedup 1.0000x reference)
import sys
sys.path.insert(0, "/opt/trn_rl_repo")
import numpy as np

import concourse.tile as tile
import concourse.bacc as bacc_mod
from concourse import bass, mybir
from concourse.bass import IndirectOffsetOnAxis
from concourse.bass_utils import run_bass_kernel_spmd
from concourse.masks import make_identity

P = 128
N, E, IN_F, OUT_F, HEADS = 100000, 1600000, 128, 32, 6
NCORES = 8
NPC = N // NCORES            # nodes per core
NBLK = (NPC + P - 1) // P    # dst blocks per core
RB = 256                     # bf16 slots per table row (512B)
ADW = 16                     # f32 slots per attn row (64B)
CH = 16                      # tiles per offset chunk
HC = HEADS * OUT_F           # 192
GT = 224                     # gathered slots per row (448B, 64B-aligned)
LAM, SALPHA = 1.0507009873554805, 1.6732632423543772

f32, bf16, i32 = mybir.dt.float32, mybir.dt.bfloat16, mybir.dt.int32
AF, OP = mybir.ActivationFunctionType, mybir.AluOpType

_cache = {}
TRACE = False
TRACE_DIR = None
LAST = {}


def _host_prep(x, edge_index, W, att_src, att_dst, bias):
    src = np.asarray(edge_index[0], dtype=np.int64)
    dst = np.asarray(edge_index[1], dtype=np.int64)
    W64 = np.asarray(W, dtype=np.float64)
    a_s64 = np.asarray(att_src, dtype=np.float64)
    a_d64 = np.asarray(att_dst, dtype=np.float64)
    w_s = np.stack([W64[:, h * OUT_F:(h + 1) * OUT_F] @ a_s64[h] for h in range(HEADS)], axis=1)
    w_d = np.stack([W64[:, h * OUT_F:(h + 1) * OUT_F] @ a_d64[h] for h in range(HEADS)], axis=1)
    Wcat = np.concatenate([W64, w_s, w_d], axis=1).astype(np.float32)  # [128, 204]
    Wcat = np.pad(Wcat, ((0, 0), (0, 4)))                              # [128, 208]

    import ml_dtypes
    Wcat_bf = Wcat.astype(ml_dtypes.bfloat16).view(np.uint16)
    xT_bf = np.ascontiguousarray(np.asarray(x, np.float32).T).astype(ml_dtypes.bfloat16).view(np.uint16)  # [128, N]

    core_of = dst // NPC
    order = np.argsort(dst, kind="stable")
    src_s, dst_s = src[order], dst[order]
    core_s = core_of[order]

    # per (core, block) edge counts
    ldst = dst_s - core_s * NPC
    blk = ldst // P
    cnt = np.zeros((NCORES, NBLK), dtype=np.int64)
    np.add.at(cnt, (core_s, blk), 1)
    Klist = [int(np.ceil(cnt[:, b].max() / P)) for b in range(NBLK)]
    T = sum(Klist)
    Tpad = ((T + CH - 1) // CH) * CH
    t0 = np.cumsum([0] + Klist[:-1])

    offs_src = np.zeros((NCORES, Tpad, P), dtype=np.int32)
    offs_dst = np.zeros((NCORES, Tpad, P), dtype=np.int32)
    dstl_f = np.full((NCORES, Tpad, P), 999.0, dtype=np.float32)

    cstart = np.searchsorted(core_s, np.arange(NCORES + 1))
    for c in range(NCORES):
        e0, e1 = cstart[c], cstart[c + 1]
        sc, lc, bc = src_s[e0:e1], ldst[e0:e1], blk[e0:e1]
        bstart = np.searchsorted(bc, np.arange(NBLK + 1))
        for b in range(NBLK):
            i0, i1 = bstart[b], bstart[b + 1]
            n = i1 - i0
            base = t0[b]
            s_pad = np.zeros(Klist[b] * P, dtype=np.int32)
            d_pad = np.zeros(Klist[b] * P, dtype=np.int32)
            l_pad = np.full(Klist[b] * P, 999.0, dtype=np.float32)
            s_pad[:n] = sc[i0:i1]
            d_pad[:n] = sc[i0:i1] * 0 + (lc[i0:i1] + c * NPC)  # global dst
            l_pad[:n] = (lc[i0:i1] - b * P).astype(np.float32)
            offs_src[c, base:base + Klist[b]] = s_pad.reshape(Klist[b], P)
            offs_dst[c, base:base + Klist[b]] = d_pad.reshape(Klist[b], P)
            dstl_f[c, base:base + Klist[b]] = l_pad.reshape(Klist[b], P)

    # chunked transposed layouts: [nch, P, CH]
    nch = Tpad // CH
    def chunkT(a):
        return np.ascontiguousarray(a.reshape(NCORES, nch, CH, P).transpose(0, 1, 3, 2))
    offs_srcT, offs_dstT = chunkT(offs_src), chunkT(offs_dst)
    dstlT = chunkT(dstl_f).astype(ml_dtypes.bfloat16).view(np.uint16)

    nbc = (NBLK + CH - 1) // CH
    oblk = np.zeros((NCORES, nbc * CH, P), dtype=np.int32)
    for c in range(NCORES):
        for b in range(NBLK):
            rows = c * NPC + b * P + np.arange(P)
            rows = np.minimum(rows, c * NPC + NPC - 1)
            oblk[c, b] = rows
    oblkT = np.ascontiguousarray(oblk.reshape(NCORES, nbc, CH, P).transpose(0, 1, 3, 2))
    iota_f = np.ascontiguousarray(np.broadcast_to(
        np.arange(P, dtype=np.float32)[None, :], (P, P))).astype(
        ml_dtypes.bfloat16).view(np.uint16)
    bias_rep = np.broadcast_to(np.asarray(bias, np.float32)[None, :], (P, OUT_F)).copy()

    in_maps = []
    for c in range(NCORES):
        in_maps.append({
            "xT": xT_bf,
            "wcat": Wcat_bf,
            "osrc": offs_srcT[c],
            "odst": offs_dstT[c],
            "dstl": dstlT[c],
            "oblk": oblkT[c],
            "iota": iota_f,
            "biasr": bias_rep,
        })
    return in_maps, Klist, nch


def _build(Klist, nch):
    nc = bacc_mod.Bacc("TRN2")
    t_xT = nc.dram_tensor("xT", [P, N], bf16, kind="ExternalInput")
    t_wc = nc.dram_tensor("wcat", [P, 208], bf16, kind="ExternalInput")
    t_osrc = nc.dram_tensor("osrc", [nch, P, CH], i32, kind="ExternalInput")
    t_odst = nc.dram_tensor("odst", [nch, P, CH], i32, kind="ExternalInput")
    t_dstl = nc.dram_tensor("dstl", [nch, P, CH], bf16, kind="ExternalInput")
    t_oblk = nc.dram_tensor("oblk", [(NBLK + CH - 1) // CH, P, CH], i32, kind="ExternalInput")
    t_iota = nc.dram_tensor("iota", [P, P], bf16, kind="ExternalInput")
    t_bias = nc.dram_tensor("biasr", [P, OUT_F], f32, kind="ExternalInput")
    t_out = nc.dram_tensor("out", [NPC, OUT_F], f32, kind="ExternalOutput")
    t_tab = nc.dram_tensor("tab", [N + 1, RB], bf16)

    NT_A = (N + P - 1) // P

    with tile.TileContext(nc) as tc:
        # ---------------- phase A: tab[n] = [xp bf16(192) | a_s f32 @96 | a_d f32 @102]
        with tc.tile_pool(name="acons", bufs=1) as acons, \
             tc.tile_pool(name="asb", bufs=4) as asb, \
             tc.tile_pool(name="aps", bufs=4, space="PSUM") as aps:
            wc_t = acons.tile([P, 208], bf16)
            nc.sync.dma_start(out=wc_t[:], in_=t_wc[:, :])
            G4 = 4
            for g in range((NT_A + G4 - 1) // G4):
                c0 = g * G4 * P
                w = min(G4 * P, N - c0)
                ntile = (w + P - 1) // P
                xt = asb.tile([P, G4 * P], bf16, tag="xt")
                nc.sync.dma_start(out=xt[:, 0:w], in_=t_xT[:, c0:c0 + w])
                row = asb.tile([P, G4 * RB], bf16, tag="row")
                rowf = row[:].bitcast(f32)
                for i in range(ntile):
                    m = min(P, w - i * P)
                    pj = aps.tile([P, 208], f32, space="PSUM", tag="pj")
                    nc.tensor.matmul(out=pj[0:m, :], lhsT=xt[:, i * P:i * P + m],
                                     rhs=wc_t[:], start=True, stop=True)
                    nc.vector.tensor_copy(out=row[0:m, i * RB:i * RB + HC],
                                          in_=pj[0:m, 0:HC])
                    nc.vector.tensor_copy(out=rowf[0:m, i * 128 + 96:i * 128 + 112],
                                          in_=pj[0:m, HC:HC + 16])
                nfull = w // P
                if nfull:
                    nc.sync.dma_start(
                        out=t_tab[c0:c0 + nfull * P, 0:GT].rearrange("(i p) r -> p i r", p=P),
                        in_=row[:, :].rearrange("p (i r) -> p i r", r=RB)[:, 0:nfull, 0:GT])
                if w % P:
                    m = w % P
                    nc.sync.dma_start(
                        out=t_tab[c0 + nfull * P:c0 + w, 0:GT],
                        in_=row[0:m, nfull * RB:nfull * RB + GT])

        # ---------------- phase B
        with tc.tile_pool(name="bcons", bufs=1) as bcons, \
             tc.tile_pool(name="bsb", bufs=3) as bsb, \
             tc.tile_pool(name="gsb", bufs=6) as gsb, \
             tc.tile_pool(name="bps", bufs=2, space="PSUM") as bps:
            iota_t = bcons.tile([P, P], bf16)
            nc.sync.dma_start(out=iota_t[:], in_=t_iota[:, :])
            bias_t = bcons.tile([P, OUT_F], f32)
            nc.sync.dma_start(out=bias_t[:], in_=t_bias[:, :])
            ident = bcons.tile([P, P], bf16)
            make_identity(nc, ident[:])
            hm_all = bcons.tile([P, NBLK * OUT_F], f32)

            t = 0
            cur = {}
            for b in range(len(Klist)):
                r0 = b * P
                m = min(P, NPC - r0)
                acc = bps.tile([P, HEADS + HC], f32, space="PSUM", tag="acc")
                if b % CH == 0:
                    oblk_t = bsb.tile([P, CH], i32, tag="oblk")
                    nc.sync.dma_start(out=oblk_t[:], in_=t_oblk[b // CH, :, :])
                ADB = bsb.tile([P, GT], bf16, tag="ADB")
                nc.gpsimd.indirect_dma_start(
                    out=ADB[:, :], out_offset=None, in_=t_tab[:, :],
                    in_offset=IndirectOffsetOnAxis(ap=oblk_t[:, b % CH:b % CH + 1], axis=0))
                adb_b = bsb.tile([P, HEADS], bf16, tag="adb_b")
                nc.vector.tensor_copy(out=adb_b[:], in_=ADB[:].bitcast(f32)[:, 102:108])
                for k in range(Klist[b]):
                    g, o = t // CH, t % CH
                    if o == 0:
                        osrc_t = bsb.tile([P, CH], i32, tag="osrc")
                        nc.sync.dma_start(out=osrc_t[:], in_=t_osrc[g, :, :])
                        odst_t = bsb.tile([P, CH], i32, tag="odst")
                        nc.sync.dma_start(out=odst_t[:], in_=t_odst[g, :, :])
                        dstl_t = bsb.tile([P, CH], bf16, tag="dstl")
                        nc.sync.dma_start(out=dstl_t[:], in_=t_dstl[g, :, :])
                        cur = {"s": osrc_t, "d": odst_t, "l": dstl_t}
                    G = gsb.tile([P, GT], bf16, tag="G")
                    nc.gpsimd.indirect_dma_start(
                        out=G[:, :], out_offset=None, in_=t_tab[:, :],
                        in_offset=IndirectOffsetOnAxis(ap=cur["s"][:, o:o + 1], axis=0))
                    S = bsb.tile([P, P], bf16, tag="S")
                    nc.vector.tensor_tensor(out=S[:], in0=iota_t[:],
                                            in1=cur["l"][:, o:o + 1].to_broadcast([P, P]),
                                            op=OP.is_equal)
                    ST_ps = bps.tile([P, P], bf16, space="PSUM", tag="ST_ps")
                    nc.tensor.transpose(out=ST_ps[:], in_=S[:], identity=ident[:])
                    ST = bsb.tile([P, P], bf16, tag="ST")
                    nc.vector.tensor_copy(out=ST[:], in_=ST_ps[:])
                    adx_ps = bps.tile([P, HEADS], f32, space="PSUM", tag="adx_ps")
                    nc.tensor.matmul(out=adx_ps[:], lhsT=ST[:], rhs=adb_b[:],
                                     start=True, stop=True)
                    gf = G[:].bitcast(f32)
                    logit = bsb.tile([P, HEADS], f32, tag="logit")
                    nc.vector.tensor_tensor(out=logit[:], in0=gf[:, 96:102],
                                            in1=adx_ps[:], op=OP.add)
                    p_t = bsb.tile([P, HEADS], f32, tag="p")
                    nc.scalar.activation(out=p_t[:], in_=logit[:], func=AF.Exp)
                    msgf = bsb.tile([P, HEADS + HC], bf16, tag="msgf")
                    nc.scalar.activation(out=msgf[:, 0:HEADS], in_=logit[:],
                                         func=AF.Exp, scale=0.2)
                    nc.vector.tensor_tensor(out=msgf[:, 0:HEADS], in0=msgf[:, 0:HEADS],
                                            in1=p_t[:], op=OP.max)
                    nc.vector.tensor_tensor(
                        out=msgf[:, HEADS:].rearrange("p (h c) -> p h c", h=HEADS),
                        in0=G[:, 0:HC].rearrange("p (h c) -> p h c", h=HEADS),
                        in1=msgf[:, 0:HEADS].unsqueeze(2).to_broadcast([P, HEADS, OUT_F]),
                        op=OP.mult)
                    nc.tensor.matmul(out=acc[:], lhsT=S[:], rhs=msgf[:],
                                     start=(k == 0), stop=(k == Klist[b] - 1))
                    t += 1
                # normalize + head mean + bias + selu
                s_eps = bsb.tile([P, HEADS], f32, tag="s_eps")
                nc.vector.tensor_scalar(out=s_eps[:], in0=acc[:, 0:HEADS],
                                        scalar1=float(HEADS), scalar2=1e-16,
                                        op0=OP.mult, op1=OP.add)
                r_t = bsb.tile([P, HEADS], f32, tag="r_t")
                nc.vector.reciprocal(out=r_t[:], in_=s_eps[:])
                out_n = bsb.tile([P, HC], f32, tag="out_n")
                nc.vector.tensor_tensor(
                    out=out_n[:].rearrange("p (h c) -> p h c", h=HEADS),
                    in0=acc[:, HEADS:],
                    in1=r_t[:].unsqueeze(2).to_broadcast([P, HEADS, OUT_F]),
                    op=OP.mult)
                nc.vector.tensor_reduce(out=hm_all[:, b * OUT_F:(b + 1) * OUT_F],
                                        in_=out_n[:].rearrange("p (h c) -> p c h", h=HEADS),
                                        axis=mybir.AxisListType.X, op=OP.add)
            W_ALL = NBLK * OUT_F
            nc.vector.tensor_tensor(
                out=hm_all[:].rearrange("p (b c) -> p b c", c=OUT_F),
                in0=hm_all[:].rearrange("p (b c) -> p b c", c=OUT_F),
                in1=bias_t[:].unsqueeze(1).to_broadcast([P, NBLK, OUT_F]),
                op=OP.add)
            pos = bcons.tile([P, W_ALL], f32)
            nc.vector.tensor_scalar(out=pos[:], in0=hm_all[:], scalar1=0.0,
                                    scalar2=None, op0=OP.max)
            neg = bcons.tile([P, W_ALL], f32)
            nc.vector.tensor_scalar(out=neg[:], in0=hm_all[:], scalar1=0.0,
                                    scalar2=None, op0=OP.min)
            nc.scalar.activation(out=neg[:], in_=neg[:], func=AF.Exp)
            nc.vector.tensor_scalar(out=neg[:], in0=neg[:], scalar1=LAM * SALPHA,
                                    scalar2=-LAM * SALPHA, op0=OP.mult, op1=OP.add)
            nc.vector.tensor_scalar(out=pos[:], in0=pos[:], scalar1=LAM,
                                    scalar2=None, op0=OP.mult)
            nc.vector.tensor_tensor(out=pos[:], in0=pos[:], in1=neg[:], op=OP.add)
            nfull_b = NPC // P
            nc.sync.dma_start(
                out=t_out[0:nfull_b * P, :].rearrange("(b p) c -> p b c", p=P),
                in_=pos[:, 0:nfull_b * OUT_F].rearrange("p (b c) -> p b c", c=OUT_F))
            mlast = NPC - nfull_b * P
            nc.sync.dma_start(
                out=t_out[nfull_b * P:NPC, :],
                in_=pos[0:mlast, nfull_b * OUT_F:(nfull_b + 1) * OUT_F])

    nc.compile()
    return nc


def kernel(x, edge_index, W, att_src, att_dst, bias):
    in_maps, Klist, nch = _host_prep(x, edge_index, W, att_src, att_dst, bias)
    key = (tuple(Klist), nch)
    if key not in _cache:
        _cache[key] = _build(Klist, nch)
    nc = _cache[key]
    res = run_bass_kernel_spmd(nc, in_maps, core_ids=list(range(NCORES)), trace=TRACE,
                               tmpdir=TRACE_DIR)
    LAST["res"] = res
    out = np.concatenate([np.asarray(res.results[c]["out"]) for c in range(NCORES)], axis=0)
    return out.astype(np.float32)


if __name__ == "__main__":
    pass

